# revision 15
# baseline (speedup 1.0000x reference)
"""VQ codebook lookup kernel for Trainium2 (8 NeuronCores, SPMD data-parallel).

Problem: z [16384, 256] f32, codebook [8192, 256] f32 ->
  tokens  [16384] i32  = argmin_v ||z - cb[v]||^2
  z_q     [16384,256]  = cb[tokens]
  ref_cnt [8192]  f32  = histogram(tokens)

Sharding: z split into 8 shards of 2048 rows (one per core); codebook
replicated; per-core partial ref_count summed on host (the all-reduce).

Device algorithm per core (2048 rows = 16 m-tiles of 128 partitions):
  argmin_v dist = argmax_v H,  H[n,v] = z.cb[v] - 0.5*||cb[v]||^2
  (row-constant ||z||^2 dropped).
  - PE: H via fp16 two-term split matmuls (z = z_hi + z_lo, cb likewise;
    hi*hi + hi*lo + lo*hi accumulated in fp32 PSUM; lo*lo ~ 1e-6, dropped)
    plus a K=2 bias matmul adding -0.5*e_sq (fp16 hi+lo rows).
  - ACT: copy PSUM quarters -> SBUF H [128, 8192] f32.
  - DVE: chunk-max reduce [128, 64, 128] -> Mc; max8 + max_index over Mc
    gives M (row max) and c* (winning 128-chunk); one scalar_tensor_tensor
    pass (H == M) * iota_local, accum -> p (position in chunk).
    token = 128*c* + p.
  - GPSIMD: indirect-DMA gather cb[token] -> z_q rows.
  - PE tail: ref_count = sum_n onehot64(c*)^T @ onehot128(p) accumulated
    over m-tiles in one [64, 128] PSUM tile -> [8192].
"""
import sys

sys.path.insert(0, "/opt/trn_rl_repo")

import numpy as np
import concourse.bass as bass
import concourse.bacc as bacc
import concourse.tile as tile
from concourse import mybir
from concourse.bass_utils import run_bass_kernel_spmd

N_CORES = 8
N_FULL = 16384
N_SHARD = N_FULL // N_CORES  # 2048
E = 256
V = 8192
P = 128
M_TILES = N_SHARD // P  # 16
N_CHUNKS = V // P       # 64 chunks of 128 (for Mc)
QUARTER = 2048          # psum quarter width (4 banks)

f32 = mybir.dt.float32
f16 = mybir.dt.float16
u8 = mybir.dt.uint8
i32 = mybir.dt.int32
u32 = mybir.dt.uint32
alu = mybir.AluOpType
AXIS_X = mybir.AxisListType.X


def _gen():
    nc = bacc.Bacc()

    zT_d = nc.dram_tensor("zT", [4 * P, N_SHARD], f16, kind="ExternalInput")
    cbT_d = nc.dram_tensor("cbT", [4 * P, V], f16, kind="ExternalInput")
    bias_d = nc.dram_tensor("biasT", [2, V], f16, kind="ExternalInput")
    ones_d = nc.dram_tensor("ones2", [2, P], f16, kind="ExternalInput")
    iota8_d = nc.dram_tensor("iota8", [P, V], u8, kind="ExternalInput")
    iotaf_d = nc.dram_tensor("iotaf", [P, 192], f32, kind="ExternalInput")
    cb_d = nc.dram_tensor("cb", [V, E], f32, kind="ExternalInput")

    tok_d = nc.dram_tensor("tokens", [N_SHARD], i32, kind="ExternalOutput")
    zq_d = nc.dram_tensor("zq", [N_SHARD, E], f32, kind="ExternalOutput")
    ref_d = nc.dram_tensor("refc", [V], f32, kind="ExternalOutput")

    with tile.TileContext(nc) as tc:
        with (
            tc.tile_pool(name="const", bufs=1) as cp,
            tc.tile_pool(name="hpool", bufs=2) as hp,
            tc.tile_pool(name="spool", bufs=2) as sp,
            tc.tile_pool(name="zqp", bufs=2) as zqp,
        ):
            # ---- resident inputs ----
            zt = [cp.tile([P, N_SHARD], f16, name=f"zt{k}", tag=f"zt{k}")
                  for k in range(4)]
            cbt = [[cp.tile([P, QUARTER], f16, name=f"cbt{k}_{pc}",
                            tag=f"cbt{k}_{pc}") for pc in range(4)]
                   for k in range(4)]
            pre_bias = cp.tile([2, V], f16, tag="bias")
            pre_ones2 = cp.tile([2, P], f16, tag="ones2")
            pre_iota8 = cp.tile([P, V], u8, tag="iota8")
            pre_iotaf = cp.tile([P, 192], f32, tag="iotaf")
            nc.sync.dma_start(pre_bias[:], bias_d[:])
            nc.sync.dma_start(pre_ones2[:], ones_d[:])
            nc.sync.dma_start(pre_iota8[:], iota8_d[:])
            nc.sync.dma_start(pre_iotaf[:], iotaf_d[:])
            for k in range(4):
                nc.sync.dma_start(zt[k][:], zT_d[k * P:(k + 1) * P, :])
            for pc in range(4):
                lo = pc * QUARTER
                for k in range(4):
                    nc.sync.dma_start(cbt[k][pc][:],
                                      cbT_d[k * P:(k + 1) * P, lo:lo + QUARTER])
            bias, ones2, iota8, iotaf = pre_bias, pre_ones2, pre_iota8, pre_iotaf

            a_all = cp.tile([P, M_TILES, 64], f16, tag="a_all")
            b_all = cp.tile([P, M_TILES, P], f16, tag="b_all")

            with tc.tile_pool(name="psum", bufs=2, space="PSUM") as pp:
                for i in range(M_TILES):
                    h = hp.tile([P, V], f32, tag="h")
                    mc = sp.tile([P, N_CHUNKS], f32, tag="mc")
                    for q in range(V // QUARTER):
                        ps = pp.tile([P, QUARTER], f32, tag="ps")
                        # weight-stationary sweep: for each lhsT, all 4 chunks
                        wsets = [
                            (ones2[:], None, 1, True, False),
                            (zt[0][:, i * P:(i + 1) * P], cbt[0][q], 0, False, False),
                            (zt[1][:, i * P:(i + 1) * P], cbt[1][q], 0, False, False),
                            (zt[0][:, i * P:(i + 1) * P], cbt[2][q], 0, False, False),
                            (zt[1][:, i * P:(i + 1) * P], cbt[3][q], 0, False, False),
                            (zt[2][:, i * P:(i + 1) * P], cbt[0][q], 0, False, False),
                            (zt[3][:, i * P:(i + 1) * P], cbt[1][q], 0, False, True),
                        ]
                        for (lhsT, rhs_t, is_bias, st, sto) in wsets:
                            for c in range(QUARTER // 512):
                                v0 = q * QUARTER + c * 512
                                rhs = (rhs_t[:, c * 512:(c + 1) * 512]
                                       if not is_bias
                                       else bias[:, v0:v0 + 512])
                                nc.tensor.matmul(
                                    ps[:, c * 512:(c + 1) * 512],
                                    lhsT=lhsT,
                                    rhs=rhs,
                                    start=st,
                                    stop=sto,
                                )
                        nc.scalar.copy(h[:, q * QUARTER:(q + 1) * QUARTER], ps[:])
                        # per-quarter chunk-max from the SBUF copy
                        nq = QUARTER // P
                        nc.vector.tensor_reduce(
                            mc[:, q * nq:(q + 1) * nq],
                            h[:, q * QUARTER:(q + 1) * QUARTER].rearrange(
                                "p (c j) -> p c j", j=P),
                            axis=AXIS_X,
                            op=alu.max,
                        )

                    # ---- argmax ----
                    top8 = sp.tile([P, 8], f32, tag="top8")
                    idx8 = sp.tile([P, 8], u32, tag="idx8")
                    nc.vector.max(out=top8[:], in_=mc[:])
                    nc.vector.max_index(out=idx8[:], in_max=top8[:], in_values=mc[:])
                    cstar_f = sp.tile([P, 1], f32, tag="cstar_f")
                    nc.vector.tensor_copy(cstar_f[:], idx8[:, 0:1])

                    scratch = sp.tile([P, V], u8, tag="scratch")
                    p_a = sp.tile([P, 1], f32, tag="p_a")
                    p_b = sp.tile([P, 1], f32, tag="p_b")
                    HALF = V // 2
                    nc.vector.scalar_tensor_tensor(
                        out=scratch[:, 0:HALF],
                        in0=h[:, 0:HALF],
                        scalar=top8[:, 0:1],
                        in1=iota8[:, 0:HALF],
                        op0=alu.is_equal,
                        op1=alu.mult,
                        accum_out=p_a[:],
                    )
                    nc.vector.scalar_tensor_tensor(
                        out=scratch[:, HALF:V],
                        in0=h[:, HALF:V],
                        scalar=top8[:, 0:1],
                        in1=iota8[:, HALF:V],
                        op0=alu.is_equal,
                        op1=alu.mult,
                        accum_out=p_b[:],
                    )
                    p_f = sp.tile([P, 1], f32, tag="p_f")
                    nc.vector.tensor_add(p_f[:], p_a[:], p_b[:])

                    tok_f = sp.tile([P, 1], f32, tag="tok_f")
                    nc.vector.scalar_tensor_tensor(
                        out=tok_f[:],
                        in0=cstar_f[:],
                        scalar=128.0,
                        in1=p_f[:],
                        op0=alu.mult,
                        op1=alu.add,
                    )
                    tok_i = sp.tile([P, 1], i32, tag="tok_i")
                    nc.vector.tensor_copy(tok_i[:], tok_f[:])
                    nc.scalar.dma_start(tok_d[i * P:(i + 1) * P, None], tok_i[:])

                    # one-hots for the ref_count outer-product matmul
                    nc.vector.tensor_scalar(
                        a_all[:, i, :], iotaf[:, 0:64], cstar_f[:], None,
                        op0=alu.is_equal,
                    )
                    nc.vector.tensor_scalar(
                        b_all[:, i, :], iotaf[:, 64:192], p_f[:], None,
                        op0=alu.is_equal,
                    )

                    # z_q gather + writeback
                    zq_t = zqp.tile([P, E], f32, tag="zq_t")
                    nc.gpsimd.indirect_dma_start(
                        out=zq_t[:],
                        out_offset=None,
                        in_=cb_d[:],
                        in_offset=bass.IndirectOffsetOnAxis(ap=tok_i[:, 0:1], axis=0),
                    )
                    nc.gpsimd.dma_start(zq_d[i * P:(i + 1) * P, :], zq_t[:])

            # ---- tail: ref_count ----
            with tc.tile_pool(name="psum2", bufs=1, space="PSUM") as pp2:
                ps_cnt = pp2.tile([64, P], f32, tag="ps_cnt")
                for i in range(M_TILES):
                    nc.tensor.matmul(
                        ps_cnt[:],
                        lhsT=a_all[:, i, :],
                        rhs=b_all[:, i, :],
                        start=(i == 0),
                        stop=(i == M_TILES - 1),
                    )
                refc_sb = sp.tile([64, P], f32, tag="refc_sb")
                nc.scalar.copy(refc_sb[:], ps_cnt[:])
                nc.scalar.dma_start(
                    ref_d[:].rearrange("(a b) -> a b", b=P), refc_sb[:]
                )

    nc.compile()
    return nc


_NC = None
LAST_RESULTS = None


def _get_nc():
    global _NC
    if _NC is None:
        _NC = _gen()
    return _NC


def _split_f16(x):
    hi = x.astype(np.float16)
    lo = (x - hi.astype(np.float32)).astype(np.float16)
    return hi, lo


def kernel(z, codebook):
    z = np.asarray(z, dtype=np.float32)
    cb = np.asarray(codebook, dtype=np.float32)

    zh, zl = _split_f16(z)
    ch, cl = _split_f16(cb)
    # k-tile layout: [hi_k0, hi_k1, lo_k0, lo_k1] along the 4*128 dim
    cbT = np.concatenate([ch.T, cl.T], axis=0)  # [512, 8192] f16
    e_sq = (cb.astype(np.float64) ** 2).sum(1)
    b = (-0.5 * e_sq).astype(np.float32)
    bh = b.astype(np.float16)
    bl = (b - bh.astype(np.float32)).astype(np.float16)
    biasT = np.stack([bh, bl], axis=0)  # [2, 8192] f16
    ones2 = np.ones((2, P), dtype=np.float16)
    iota8 = np.broadcast_to(
        (np.arange(V, dtype=np.uint16) % P).astype(np.uint8), (P, V)
    ).copy()
    iotaf = np.broadcast_to(
        np.concatenate([np.arange(64), np.arange(128)]).astype(np.float32), (P, 192)
    ).copy()

    in_maps = []
    for i in range(N_CORES):
        zs_h = zh[i * N_SHARD:(i + 1) * N_SHARD]
        zs_l = zl[i * N_SHARD:(i + 1) * N_SHARD]
        zT = np.concatenate([zs_h.T, zs_l.T], axis=0)  # [512, 2048] f16
        in_maps.append({
            "zT": np.ascontiguousarray(zT),
            "cbT": np.ascontiguousarray(cbT),
            "biasT": biasT,
            "ones2": ones2,
            "iota8": iota8,
            "iotaf": iotaf,
            "cb": cb,
        })

    import os
    try:
        res = run_bass_kernel_spmd(
            _get_nc(), in_maps, core_ids=list(range(N_CORES)),
            trace=bool(os.environ.get("VQ_TRACE")),
        )
    except Exception as e:
        print(f'trace attempt failed ({type(e).__name__}: {e}); rerunning untraced')
        res = run_bass_kernel_spmd(
            _get_nc(), in_maps, core_ids=list(range(N_CORES)),
        )
    global LAST_RESULTS
    LAST_RESULTS = res

    tokens = np.concatenate([r["tokens"].reshape(-1) for r in res.results])
    z_q = np.concatenate([r["zq"] for r in res.results], axis=0)
    ref_count = np.sum([r["refc"].reshape(-1) for r in res.results], axis=0,
                       dtype=np.float32)
    return tokens.astype(np.int32), z_q.astype(np.float32), ref_count


if __name__ == "__main__":
    rng = np.random.default_rng(0)
    z = rng.standard_normal((N_FULL, E)).astype(np.float32)
    cb = rng.standard_normal((V, E)).astype(np.float32)
    t, q, r = kernel(z, cb)
    print(t[:8], q.shape, r.sum())


# revision 16
# speedup vs baseline: 1.0364x; 1.0364x over previous
"""VQ codebook lookup kernel for Trainium2 (8 NeuronCores, SPMD data-parallel).

Problem: z [16384, 256] f32, codebook [8192, 256] f32 ->
  tokens  [16384] i32  = argmin_v ||z - cb[v]||^2
  z_q     [16384,256]  = cb[tokens]
  ref_cnt [8192]  f32  = histogram(tokens)

Sharding: z split into 8 shards of 2048 rows (one per core); codebook
replicated; per-core partial ref_count summed on host (the all-reduce).

Device algorithm per core (2048 rows = 16 m-tiles of 128 partitions):
  argmin_v dist = argmax_v H,  H[n,v] = z.cb[v] - 0.5*||cb[v]||^2
  (row-constant ||z||^2 dropped).
  - PE: H via fp16 two-term split matmuls (z = z_hi + z_lo, cb likewise;
    hi*hi + hi*lo + lo*hi accumulated in fp32 PSUM; lo*lo ~ 1e-6, dropped)
    plus a K=2 bias matmul adding -0.5*e_sq (fp16 hi+lo rows).
  - ACT: copy PSUM quarters -> SBUF H [128, 8192] f32.
  - DVE: chunk-max reduce [128, 64, 128] -> Mc; max8 + max_index over Mc
    gives M (row max) and c* (winning 128-chunk); one scalar_tensor_tensor
    pass (H == M) * iota_local, accum -> p (position in chunk).
    token = 128*c* + p.
  - GPSIMD: indirect-DMA gather cb[token] -> z_q rows.
  - PE tail: ref_count = sum_n onehot64(c*)^T @ onehot128(p) accumulated
    over m-tiles in one [64, 128] PSUM tile -> [8192].
"""
import sys

sys.path.insert(0, "/opt/trn_rl_repo")

import numpy as np
import concourse.bass as bass
import concourse.bacc as bacc
import concourse.tile as tile
from concourse import mybir
from concourse.bass_utils import run_bass_kernel_spmd

N_CORES = 8
N_FULL = 16384
N_SHARD = N_FULL // N_CORES  # 2048
E = 256
V = 8192
P = 128
M_TILES = N_SHARD // P  # 16
N_CHUNKS = V // P       # 64 chunks of 128 (for Mc)
QUARTER = 2048          # psum quarter width (4 banks)

f32 = mybir.dt.float32
f16 = mybir.dt.float16
u8 = mybir.dt.uint8
i32 = mybir.dt.int32
u32 = mybir.dt.uint32
alu = mybir.AluOpType
AXIS_X = mybir.AxisListType.X


def _gen():
    nc = bacc.Bacc()

    zT_d = nc.dram_tensor("zT", [4 * P, N_SHARD], f16, kind="ExternalInput")
    cbT_d = nc.dram_tensor("cbT", [4 * P, V], f16, kind="ExternalInput")
    bias_d = nc.dram_tensor("biasT", [2, V], f16, kind="ExternalInput")
    ones_d = nc.dram_tensor("ones2", [2, P], f16, kind="ExternalInput")
    iota8_d = nc.dram_tensor("iota8", [P, V], u8, kind="ExternalInput")
    iotaf_d = nc.dram_tensor("iotaf", [P, 192], f32, kind="ExternalInput")
    cb_d = nc.dram_tensor("cb", [V, E], f32, kind="ExternalInput")

    tok_d = nc.dram_tensor("tokens", [N_SHARD], i32, kind="ExternalOutput")
    zq_d = nc.dram_tensor("zq", [N_SHARD, E], f32, kind="ExternalOutput")
    ref_d = nc.dram_tensor("refc", [V], f32, kind="ExternalOutput")

    with tile.TileContext(nc) as tc:
        with (
            tc.tile_pool(name="const", bufs=1) as cp,
            tc.tile_pool(name="hpool", bufs=2) as hp,
            tc.tile_pool(name="spool", bufs=2) as sp,
            tc.tile_pool(name="zqp", bufs=2) as zqp,
        ):
            # ---- resident inputs ----
            zt = [cp.tile([P, N_SHARD], f16, name=f"zt{k}", tag=f"zt{k}")
                  for k in range(4)]
            cbt = [[cp.tile([P, QUARTER], f16, name=f"cbt{k}_{pc}",
                            tag=f"cbt{k}_{pc}") for pc in range(4)]
                   for k in range(4)]
            pre_bias = cp.tile([2, V], f16, tag="bias")
            pre_ones2 = cp.tile([2, P], f16, tag="ones2")
            pre_iota8 = cp.tile([P, V], u8, tag="iota8")
            pre_iotaf = cp.tile([P, 192], f32, tag="iotaf")
            nc.sync.dma_start(pre_bias[:], bias_d[:])
            nc.sync.dma_start(pre_ones2[:], ones_d[:])
            nc.sync.dma_start(pre_iota8[:], iota8_d[:])
            nc.sync.dma_start(pre_iotaf[:], iotaf_d[:])
            for k in range(4):
                nc.sync.dma_start(zt[k][:], zT_d[k * P:(k + 1) * P, :])
            for pc in range(4):
                lo = pc * QUARTER
                for k in range(4):
                    nc.sync.dma_start(cbt[k][pc][:],
                                      cbT_d[k * P:(k + 1) * P, lo:lo + QUARTER])
            bias, ones2, iota8, iotaf = pre_bias, pre_ones2, pre_iota8, pre_iotaf

            a_all = cp.tile([P, M_TILES, 64], f16, tag="a_all")
            b_all = cp.tile([P, M_TILES, P], f16, tag="b_all")

            with tc.tile_pool(name="psum", bufs=2, space="PSUM") as pp:
                for i in range(M_TILES):
                    h = hp.tile([P, V], f32, tag="h")
                    mc = sp.tile([P, N_CHUNKS], f32, tag="mc")
                    for q in range(V // QUARTER):
                        ps = pp.tile([P, QUARTER], f32, tag="ps")
                        # weight-stationary sweep: for each lhsT, all 4 chunks
                        wsets = [
                            (ones2[:], None, 1, True, False),
                            (zt[0][:, i * P:(i + 1) * P], cbt[0][q], 0, False, False),
                            (zt[1][:, i * P:(i + 1) * P], cbt[1][q], 0, False, False),
                            (zt[0][:, i * P:(i + 1) * P], cbt[2][q], 0, False, False),
                            (zt[1][:, i * P:(i + 1) * P], cbt[3][q], 0, False, False),
                            (zt[2][:, i * P:(i + 1) * P], cbt[0][q], 0, False, False),
                            (zt[3][:, i * P:(i + 1) * P], cbt[1][q], 0, False, True),
                        ]
                        for (lhsT, rhs_t, is_bias, st, sto) in wsets:
                            for c in range(QUARTER // 512):
                                v0 = q * QUARTER + c * 512
                                rhs = (rhs_t[:, c * 512:(c + 1) * 512]
                                       if not is_bias
                                       else bias[:, v0:v0 + 512])
                                nc.tensor.matmul(
                                    ps[:, c * 512:(c + 1) * 512],
                                    lhsT=lhsT,
                                    rhs=rhs,
                                    start=st,
                                    stop=sto,
                                )
                        nc.scalar.copy(h[:, q * QUARTER:(q + 1) * QUARTER], ps[:])
                        # per-quarter chunk-max from the SBUF copy
                        nq = QUARTER // P
                        nc.vector.tensor_reduce(
                            mc[:, q * nq:(q + 1) * nq],
                            h[:, q * QUARTER:(q + 1) * QUARTER].rearrange(
                                "p (c j) -> p c j", j=P),
                            axis=AXIS_X,
                            op=alu.max,
                        )

                    # ---- per-half argmax: half A's extraction runs
                    # while PE still streams half B's quarters ----
                    HALF = V // 2
                    HC = N_CHUNKS // 2
                    scratch = sp.tile([P, V], u8, tag="scratch")
                    tops = []
                    cfs = []
                    pfs = []
                    for hx, (c0, v0) in enumerate([(0, 0), (HC, HALF)]):
                        t8 = sp.tile([P, 8], f32, name=f"t8_{hx}", tag=f"t8_{hx}")
                        i8 = sp.tile([P, 8], u32, name=f"i8_{hx}", tag=f"i8_{hx}")
                        nc.vector.max(out=t8[:], in_=mc[:, c0:c0 + HC])
                        nc.vector.max_index(out=i8[:], in_max=t8[:],
                                            in_values=mc[:, c0:c0 + HC])
                        cf = sp.tile([P, 1], f32, name=f"cf_{hx}", tag=f"cf_{hx}")
                        nc.vector.tensor_copy(cf[:], i8[:, 0:1])
                        pf = sp.tile([P, 1], f32, name=f"pf_{hx}", tag=f"pf_{hx}")
                        nc.vector.scalar_tensor_tensor(
                            out=scratch[:, v0:v0 + HALF],
                            in0=h[:, v0:v0 + HALF],
                            scalar=t8[:, 0:1],
                            in1=iota8[:, v0:v0 + HALF],
                            op0=alu.is_equal,
                            op1=alu.mult,
                            accum_out=pf[:],
                        )
                        tops.append(t8)
                        cfs.append(cf)
                        pfs.append(pf)

                    # ge = 1{maxA >= maxB}; picks half A on exact ties,
                    # matching argmin first-occurrence. Selects are done
                    # arithmetically: x = ge*(xA - xB) + xB.
                    ge = sp.tile([P, 1], f32, tag="ge")
                    nc.vector.tensor_scalar(
                        ge[:], tops[0][:, 0:1], tops[1][:, 0:1], None,
                        op0=alu.is_ge,
                    )
                    cb_g = sp.tile([P, 1], f32, tag="cb_g")
                    nc.vector.tensor_scalar_add(cb_g[:], cfs[1][:], float(HC))
                    cd = sp.tile([P, 1], f32, tag="cd")
                    nc.vector.tensor_sub(cd[:], cfs[0][:], cb_g[:])
                    cstar_f = sp.tile([P, 1], f32, tag="cstar_f")
                    nc.vector.scalar_tensor_tensor(
                        out=cstar_f[:], in0=cd[:], scalar=ge[:, 0:1],
                        in1=cb_g[:], op0=alu.mult, op1=alu.add,
                    )
                    pd = sp.tile([P, 1], f32, tag="pd")
                    nc.vector.tensor_sub(pd[:], pfs[0][:], pfs[1][:])
                    p_f = sp.tile([P, 1], f32, tag="p_f")
                    nc.vector.scalar_tensor_tensor(
                        out=p_f[:], in0=pd[:], scalar=ge[:, 0:1],
                        in1=pfs[1][:], op0=alu.mult, op1=alu.add,
                    )

                    tok_f = sp.tile([P, 1], f32, tag="tok_f")
                    nc.vector.scalar_tensor_tensor(
                        out=tok_f[:],
                        in0=cstar_f[:],
                        scalar=128.0,
                        in1=p_f[:],
                        op0=alu.mult,
                        op1=alu.add,
                    )
                    tok_i = sp.tile([P, 1], i32, tag="tok_i")
                    nc.vector.tensor_copy(tok_i[:], tok_f[:])
                    nc.scalar.dma_start(tok_d[i * P:(i + 1) * P, None], tok_i[:])

                    # one-hots for the ref_count outer-product matmul
                    nc.vector.tensor_scalar(
                        a_all[:, i, :], iotaf[:, 0:64], cstar_f[:], None,
                        op0=alu.is_equal,
                    )
                    nc.vector.tensor_scalar(
                        b_all[:, i, :], iotaf[:, 64:192], p_f[:], None,
                        op0=alu.is_equal,
                    )

                    # z_q gather + writeback
                    zq_t = zqp.tile([P, E], f32, tag="zq_t")
                    nc.gpsimd.indirect_dma_start(
                        out=zq_t[:],
                        out_offset=None,
                        in_=cb_d[:],
                        in_offset=bass.IndirectOffsetOnAxis(ap=tok_i[:, 0:1], axis=0),
                    )
                    nc.gpsimd.dma_start(zq_d[i * P:(i + 1) * P, :], zq_t[:])

            # ---- tail: ref_count ----
            with tc.tile_pool(name="psum2", bufs=1, space="PSUM") as pp2:
                ps_cnt = pp2.tile([64, P], f32, tag="ps_cnt")
                for i in range(M_TILES):
                    nc.tensor.matmul(
                        ps_cnt[:],
                        lhsT=a_all[:, i, :],
                        rhs=b_all[:, i, :],
                        start=(i == 0),
                        stop=(i == M_TILES - 1),
                    )
                refc_sb = sp.tile([64, P], f32, tag="refc_sb")
                nc.scalar.copy(refc_sb[:], ps_cnt[:])
                nc.scalar.dma_start(
                    ref_d[:].rearrange("(a b) -> a b", b=P), refc_sb[:]
                )

    nc.compile()
    return nc


_NC = None
LAST_RESULTS = None


def _get_nc():
    global _NC
    if _NC is None:
        _NC = _gen()
    return _NC


def _split_f16(x):
    hi = x.astype(np.float16)
    lo = (x - hi.astype(np.float32)).astype(np.float16)
    return hi, lo


def kernel(z, codebook):
    z = np.asarray(z, dtype=np.float32)
    cb = np.asarray(codebook, dtype=np.float32)

    zh, zl = _split_f16(z)
    ch, cl = _split_f16(cb)
    # k-tile layout: [hi_k0, hi_k1, lo_k0, lo_k1] along the 4*128 dim
    cbT = np.concatenate([ch.T, cl.T], axis=0)  # [512, 8192] f16
    e_sq = (cb.astype(np.float64) ** 2).sum(1)
    b = (-0.5 * e_sq).astype(np.float32)
    bh = b.astype(np.float16)
    bl = (b - bh.astype(np.float32)).astype(np.float16)
    biasT = np.stack([bh, bl], axis=0)  # [2, 8192] f16
    ones2 = np.ones((2, P), dtype=np.float16)
    iota8 = np.broadcast_to(
        (np.arange(V, dtype=np.uint16) % P).astype(np.uint8), (P, V)
    ).copy()
    iotaf = np.broadcast_to(
        np.concatenate([np.arange(64), np.arange(128)]).astype(np.float32), (P, 192)
    ).copy()

    in_maps = []
    for i in range(N_CORES):
        zs_h = zh[i * N_SHARD:(i + 1) * N_SHARD]
        zs_l = zl[i * N_SHARD:(i + 1) * N_SHARD]
        zT = np.concatenate([zs_h.T, zs_l.T], axis=0)  # [512, 2048] f16
        in_maps.append({
            "zT": np.ascontiguousarray(zT),
            "cbT": np.ascontiguousarray(cbT),
            "biasT": biasT,
            "ones2": ones2,
            "iota8": iota8,
            "iotaf": iotaf,
            "cb": cb,
        })

    import os
    try:
        res = run_bass_kernel_spmd(
            _get_nc(), in_maps, core_ids=list(range(N_CORES)),
            trace=bool(os.environ.get("VQ_TRACE")),
        )
    except Exception as e:
        print(f'trace attempt failed ({type(e).__name__}: {e}); rerunning untraced')
        res = run_bass_kernel_spmd(
            _get_nc(), in_maps, core_ids=list(range(N_CORES)),
        )
    global LAST_RESULTS
    LAST_RESULTS = res

    tokens = np.concatenate([r["tokens"].reshape(-1) for r in res.results])
    z_q = np.concatenate([r["zq"] for r in res.results], axis=0)
    ref_count = np.sum([r["refc"].reshape(-1) for r in res.results], axis=0,
                       dtype=np.float32)
    return tokens.astype(np.int32), z_q.astype(np.float32), ref_count


if __name__ == "__main__":
    rng = np.random.default_rng(0)
    z = rng.standard_normal((N_FULL, E)).astype(np.float32)
    cb = rng.standard_normal((V, E)).astype(np.float32)
    t, q, r = kernel(z, cb)
    print(t[:8], q.shape, r.sum())
